# revision 10
# baseline (speedup 1.0000x reference)
"""Trainium2 Bass kernel for nn_DifferentiableStarPlanner.

Algorithm notes (validated bitwise vs the reference in numpy):

  * The reference's open/close/pool computations never feed the returned
    tensor: the output is exactly NUM_SWEEPS Jacobi sweeps of a 9-channel
    min-plus stencil  g <- min(g, min_c(shift_c(g) + cmap_c))  with
    g0 = 1e7 everywhere except the start cell.
  * Information propagates one cell per sweep from the start cell, so only
    the bounding box of the start support, inflated by NUM_SWEEPS and clipped
    to the grid, can ever change from 1e7.  For the shipped problem that is
    rows/cols 0..112 (a 113x113 corner of the 512x512 grid).
  * Edge-replicate padding can be replaced by +inf (1e7) guard cells: at a
    true grid edge every clamped channel is either bitwise-identical to
    another channel or provably >= it (fp32 ops used are monotone), so
    dropping them never changes the min.  Same for the center channel
    (cmap >= 0).  This leaves 8 channels and static guards.
  * Per sweep only cells within t steps of the start can change, so the
    active column window grows by one per sweep (rows ride along for free:
    they live on partitions and cost nothing).

Device mapping (one NeuronCore; all 8 cores run identical replicas):

  * g [Dr, Sc] fp32 in SBUF: partition = domain row, free = storage col
    (guard cols 0 and Sc-1 hold 1e7).
  * Row shifts cannot be done by DVE (SBUF operands must start at partition
    0/32/64/96), so shifted copies are produced by the TensorEngine in
    *transpose mode* (pure routing, bit-exact): each sweep
      T1:   psT = g^T                              (PE)
      copy: gT[:, 1:1+Dr] = psT                    (ACT; gT has 1e7 row guards)
      8x:   psum[dy,dx] += T(gT row-slice, P_dx)   (PE, accumulates onto
                                                    cmap-preloaded PSUM)
    where P_dx is a circulant permutation implementing the column shift.
  * cmap for the next sweep is preloaded into the other PSUM bank set by 8
    more transpose matmuls (overlapped with the DVE phase).
  * DVE then does one 9-way strided min-reduce over the three PSUM banks
    (the center slot is a one-time-1e7 region) and one tensor_tensor min
    with g. These two instructions are the whole per-sweep DVE cost.
"""
import sys
import os
import numpy as np

for _p in ("/opt/trn_rl_repo", "/root/.axon_site/_ro/trn_rl_repo"):
    if os.path.isdir(_p) and _p not in sys.path:
        sys.path.insert(0, _p)

import concourse.bass as bass
import concourse.bacc as bacc
import concourse.mybir as mybir
from concourse import tile
from concourse.bass_utils import run_bass_kernel_spmd

F32 = mybir.dt.float32
ALU = mybir.AluOpType
AXL = mybir.AxisListType
ACTF = mybir.ActivationFunctionType

INF = np.float32(1.0e7)
OC = float(np.float32(10000.0))
EPS_F = np.float32(1e-12)
NUM_SWEEPS = 80
N_CORES = 8

# channels: (dy, dx), center excluded
CHANNELS = [(dy, dx) for dy in (-1, 0, 1) for dx in (-1, 0, 1) if not (dy == 0 and dx == 0)]


def build_program(Dr, Dc, seed_rlo, seed_rhi, seed_clo, seed_chi, r0, c0,
                  H, W, num_sweeps):
    """Build the Bass program. Domain = grid rows r0..r0+Dr-1, cols c0..c0+Dc-1.

    seed_* are the seed bounding box in *domain* coordinates (for the growing
    active window).
    Returns (nc, input_names).
    """
    Sr, Sc = Dr + 2, Dc + 2
    assert Dr <= 126 and Sc <= 128
    assert 3 * Sc <= 512  # three PSUM regions per bank

    nc = bacc.Bacc("TRN2", target_bir_lowering=False, debug=False)

    # ---- DRAM I/O ----
    # All inputs packed into one tensor -> one DMA -> one semaphore to wait on
    # (walrus rejects compute instructions waiting on multiple DMA queues).
    TOT = 7 * Sr + Dc + 3 * Sc
    d_pack = nc.dram_tensor("packed", [Sc, TOT], F32, kind="ExternalInput")
    d_out = nc.dram_tensor("out", [H, W], F32, kind="ExternalOutput")

    with tile.TileContext(nc) as tc:
        from contextlib import ExitStack
        with ExitStack() as ctx:
            sb = ctx.enter_context(tc.tile_pool(name="sb", bufs=1))
            ps = ctx.enter_context(tc.tile_pool(name="ps", bufs=1, space="PSUM"))

            # ---- SBUF tiles ----
            t_all = sb.tile([Sc, TOT], F32)
            offs = {}
            _o = 0
            for nm in ("obsT", "obsTm", "obsTp", "xcT", "xcTm", "xcTp", "ycT"):
                offs[nm] = _o
                _o += Sr
            offs["startm"] = _o
            _o += Dc
            offs["ident"], offs["permm"], offs["permp"] = _o, _o + Sc, _o + 2 * Sc
            t_in = {nm: t_all[:, offs[nm]:offs[nm] + Sr] for nm in
                    ("obsT", "obsTm", "obsTp", "xcT", "xcTm", "xcTp", "ycT")}
            t_start = t_all[0:Dr, offs["startm"]:offs["startm"] + Dc]
            t_ident = t_all[:, offs["ident"]:offs["ident"] + Sc]
            t_permm = t_all[:, offs["permm"]:offs["permm"] + Sc]
            t_permp = t_all[:, offs["permp"]:offs["permp"] + Sc]
            # DVE-owned copies of the constant matrices: matmuls that read
            # them then depend only on the DVE semaphore (walrus allows very
            # few sync waits per LDWEIGHTS).
            t_identC = sb.tile([Sc, Sc], F32)
            t_permmC = sb.tile([Sc, Sc], F32)
            t_permpC = sb.tile([Sc, Sc], F32)
            g = sb.tile([Dr, Sc], F32)
            gT = sb.tile([Sc, Sr], F32)
            red = sb.tile([Dr, Dc], F32)
            bg = sb.tile([128, W], F32)
            bias_eps = sb.tile([Sc, 1], F32)
            sq = {nm: sb.tile([Sc, Dr], F32, name=f"sq_{nm}") for nm in ("L", "R", "U", "D")}
            t_tmp = sb.tile([Sc, Dr], F32)
            t_A = {ch: sb.tile([Sc, Dr], F32, name=f"A_{ch[0]+1}{ch[1]+1}") for ch in CHANNELS}
            t_mx = {ch: sb.tile([Sc, Dr], F32, name=f"mx_{ch[0]+1}{ch[1]+1}") for ch in CHANNELS}
            t_cmapT = {ch: sb.tile([Sc, Dr], F32, name=f"cmapT_{ch[0]+1}{ch[1]+1}") for ch in CHANNELS}

            # ---- PSUM tiles: two bank sets of 3 banks + transpose scratch ----
            psum_sets = [ps.tile([Dr, 1536], F32, name="psumA"), ps.tile([Dr, 1536], F32, name="psumB")]
            psT = ps.tile([Sc, 512], F32)

            # ---- load inputs (single DMA) ----
            nc.sync.dma_start(t_all[:], d_pack.ap())
            nc.vector.tensor_copy(t_identC[:], t_ident[:])
            nc.vector.tensor_copy(t_permmC[:], t_permm[:])
            nc.vector.tensor_copy(t_permpC[:], t_permp[:])

            # ---- constants / init ----
            nc.vector.memset(bg[:], INF)
            nc.vector.memset(g[:], INF)
            nc.vector.memset(gT[:], INF)
            nc.vector.memset(bias_eps[:], EPS_F)
            # center slots (dy=0,dx=0) of both PSUM sets: one-time 1e7
            for s in range(2):
                nc.vector.memset(psum_sets[s][:, 512 + Sc:512 + 2 * Sc], INF)

            # ---- background writes (1e7 outside the domain) ----
            out_ap = d_out.ap()
            rs = 0
            bg_rows = []
            if r0 > 0:
                bg_rows.append((0, r0))
            if r0 + Dr < H:
                bg_rows.append((r0 + Dr, H))
            for lo, hi in bg_rows:
                r = lo
                while r < hi:
                    n = min(128, hi - r)
                    nc.sync.dma_start(out_ap[r:r + n, :], bg[0:n, :])
                    r += n
            if c0 > 0:
                nc.sync.dma_start(out_ap[r0:r0 + Dr, 0:c0], bg[0:Dr, 0:c0])
            if c0 + Dc < W:
                nc.sync.dma_start(out_ap[r0:r0 + Dr, c0 + Dc:W],
                                  bg[0:Dr, 0:W - c0 - Dc])

            # ---- cmap (computed in transposed orientation) ----
            rows = slice(1, 1 + Dr)     # free-dim slice: domain rows
            v = nc.vector
            # squared coordinate diffs
            v.tensor_sub(t_tmp[:], t_in["xcT"][:, rows], t_in["xcTm"][:, rows])
            v.tensor_mul(sq["L"][:], t_tmp[:], t_tmp[:])
            v.tensor_sub(t_tmp[:], t_in["xcT"][:, rows], t_in["xcTp"][:, rows])
            v.tensor_mul(sq["R"][:], t_tmp[:], t_tmp[:])
            v.tensor_sub(t_tmp[:], t_in["ycT"][:, rows], t_in["ycT"][:, 2:2 + Dr])
            v.tensor_mul(sq["U"][:], t_tmp[:], t_tmp[:])
            v.tensor_sub(t_tmp[:], t_in["ycT"][:, rows], t_in["ycT"][:, 0:Dr])
            v.tensor_mul(sq["D"][:], t_tmp[:], t_tmp[:])

            # geometric terms: sqrt(sum + EPS); mirror reference op pairing
            geo = {(-1, -1): ("L", "U"), (0, -1): ("L",), (1, -1): ("L", "D"),
                   (-1, 0): ("U",), (1, 0): ("D",),
                   (-1, 1): ("R", "U"), (0, 1): ("R",), (1, 1): ("R", "D")}
            # obstacle-neighbor quirk of the reference: (0,-1) uses nb(-1,0)
            obsnb = {(-1, -1): (-1, -1), (0, -1): (-1, 0), (1, -1): (1, -1),
                     (-1, 0): (-1, 0), (1, 0): (1, 0),
                     (-1, 1): (-1, 1), (0, 1): (0, 1), (1, 1): (1, 1)}
            obs_by_dx = {-1: "obsTm", 0: "obsT", 1: "obsTp"}
            for ch in CHANNELS:
                terms = geo[ch]
                if len(terms) == 2:
                    v.tensor_add(t_A[ch][:], sq[terms[0]][:], sq[terms[1]][:])
                    nc.scalar.activation(t_A[ch][:], t_A[ch][:], ACTF.Sqrt,
                                         bias=bias_eps[:], scale=1.0)
                else:
                    nc.scalar.activation(t_A[ch][:], sq[terms[0]][:], ACTF.Sqrt,
                                         bias=bias_eps[:], scale=1.0)
                ody, odx = obsnb[ch]
                nbt = t_in[obs_by_dx[odx]]
                v.tensor_max(t_mx[ch][:], nbt[:, 1 + ody:1 + ody + Dr],
                             t_in["obsT"][:, rows])
                v.scalar_tensor_tensor(t_cmapT[ch][:], t_mx[ch][:], OC, t_A[ch][:],
                                       op0=ALU.mult, op1=ALU.add)

            # ---- g0 = clip(INF*(1-start), 0, INF) ----
            v.tensor_scalar(g[:, 1:1 + Dc], t_start[:], -float(INF), float(INF),
                            op0=ALU.mult, op1=ALU.add)
            v.tensor_scalar_max(g[:, 1:1 + Dc], g[:, 1:1 + Dc], 0.0)

            # ---- helpers ----
            def ap3(tile_ap, col_off, dims):
                base = tile_ap[:]
                pap = list(base.ap)
                return bass.AP(base.tensor, base.offset + col_off,
                               [list(pap[0])] + [list(d) for d in dims])

            def preload(set_idx):
                # 8 cmap preload transposes; per-bank group: start on first
                # region of each bank, no stop (closed by the shift matmuls)
                for dy in (-1, 0, 1):
                    first = True
                    for dx in (-1, 0, 1):
                        if dy == 0 and dx == 0:
                            continue
                        off = (dy + 1) * 512 + (dx + 1) * Sc
                        nc.tensor.matmul(
                            psum_sets[set_idx][:, off:off + Sc],
                            lhsT=t_cmapT[(dy, dx)][:],
                            rhs=t_identC[:],
                            is_transpose=True, start=first, stop=False)
                        first = False

            perm_by_dx = {-1: t_permmC, 0: t_identC, 1: t_permpC}

            preload(0)

            # ---- sweeps ----
            for t in range(1, num_sweeps + 1):
                cur = psum_sets[(t - 1) % 2]
                # active storage-col window
                lo = max(1, seed_clo + 1 - t)
                hi = min(Dc, seed_chi + 1 + t)
                nW = hi - lo + 1

                # T1: psT = g^T
                nc.tensor.matmul(psT[:, 0:Dr], lhsT=g[:], rhs=t_identC[0:Dr, 0:Dr],
                                 is_transpose=True, start=True, stop=True)
                nc.scalar.copy(gT[:, 1:1 + Dr], psT[:, 0:Dr])

                # 8 shift-accumulate transposes
                for dy in (-1, 0, 1):
                    dxs = [dx for dx in (-1, 0, 1) if not (dy == 0 and dx == 0)]
                    for k, dx in enumerate(dxs):
                        off = (dy + 1) * 512 + (dx + 1) * Sc
                        nc.tensor.matmul(
                            cur[:, off:off + Sc],
                            lhsT=gT[:, 1 + dy:1 + dy + Dr],
                            rhs=perm_by_dx[dx][:],
                            is_transpose=True, start=False, stop=(k == len(dxs) - 1))

                # 9-way min reduce over the three banks (center slot is 1e7)
                in_ap = ap3(cur, lo, [[1, nW], [512, 3], [Sc, 3]])
                v.tensor_reduce(red[:, 0:nW], in_ap, axis=AXL.XY, op=ALU.min)
                v.tensor_tensor(g[:, lo:hi + 1], g[:, lo:hi + 1], red[:, 0:nW],
                                op=ALU.min)

                if t < num_sweeps:
                    preload(t % 2)

            # ---- write the domain ----
            nc.sync.dma_start(out_ap[r0:r0 + Dr, c0:c0 + Dc], g[:, 1:1 + Dc])

    nc.compile()
    return nc, ["packed"]


def prep_inputs(obstacles, coords, start_map, num_sweeps=NUM_SWEEPS):
    """Host-side slicing/layout prep. Returns (in_map, geometry)."""
    obs = np.asarray(obstacles, np.float32)[0, 0]
    yc = np.asarray(coords, np.float32)[0, 0]
    xc = np.asarray(coords, np.float32)[0, 1]
    s = np.asarray(start_map, np.float32)[0, 0]
    H, W = obs.shape

    ys, xs = np.nonzero(s > 0)
    assert len(ys) >= 1, "empty start_map"
    r0 = max(0, int(ys.min()) - num_sweeps)
    r1 = min(H - 1, int(ys.max()) + num_sweeps)
    c0 = max(0, int(xs.min()) - num_sweeps)
    c1 = min(W - 1, int(xs.max()) + num_sweeps)
    Dr, Dc = r1 - r0 + 1, c1 - c0 + 1
    Sr, Sc = Dr + 2, Dc + 2

    def pad_slice(a):
        ap = np.pad(a, 1, mode='edge')
        return np.ascontiguousarray(ap[r0:r0 + Sr, c0:c0 + Sc], dtype=np.float32)

    obs_p, yc_p, xc_p = pad_slice(obs), pad_slice(yc), pad_slice(xc)

    def tsh(a, dx):
        # a is [Sr, Sc] pad array; returns [Sc, Sr] with aT[j, i] = a[i, j+dx]
        at = np.ascontiguousarray(a.T)
        if dx == 0:
            return at
        out = np.empty_like(at)
        if dx == -1:
            out[1:] = at[:-1]
            out[0] = at[0]
        else:
            out[:-1] = at[1:]
            out[-1] = at[-1]
        return out

    # permm/permp: P[k, j] = 1 iff k == (j+dx) mod Sc
    permm = np.zeros((Sc, Sc), np.float32)
    permm[(np.arange(Sc) - 1) % Sc, np.arange(Sc)] = 1.0
    permp = np.zeros((Sc, Sc), np.float32)
    permp[(np.arange(Sc) + 1) % Sc, np.arange(Sc)] = 1.0

    startm = np.zeros((Sc, Dc), np.float32)
    startm[0:Dr, :] = s[r0:r1 + 1, c0:c1 + 1]
    packed = np.concatenate([
        tsh(obs_p, 0), tsh(obs_p, -1), tsh(obs_p, 1),
        tsh(xc_p, 0), tsh(xc_p, -1), tsh(xc_p, 1), tsh(yc_p, 0),
        startm, np.eye(Sc, dtype=np.float32), permm, permp,
    ], axis=1)
    in_map = {"packed": np.ascontiguousarray(packed, dtype=np.float32)}

    geom = dict(Dr=Dr, Dc=Dc, r0=r0, c0=c0, H=H, W=W,
                seed_rlo=int(ys.min()) - r0, seed_rhi=int(ys.max()) - r0,
                seed_clo=int(xs.min()) - c0, seed_chi=int(xs.max()) - c0)
    return in_map, geom


def kernel(obstacles, coords, start_map, goal_map):
    in_map, gm = prep_inputs(obstacles, coords, start_map)
    nc, _ = build_program(gm["Dr"], gm["Dc"], gm["seed_rlo"], gm["seed_rhi"],
                          gm["seed_clo"], gm["seed_chi"], gm["r0"], gm["c0"],
                          gm["H"], gm["W"], NUM_SWEEPS)
    in_maps = [in_map for _ in range(N_CORES)]
    res = run_bass_kernel_spmd(nc, in_maps, core_ids=list(range(N_CORES)))
    out = res.results[0]["out"]
    return np.ascontiguousarray(out[None, None]).astype(np.float32)
